# revision 28
# baseline (speedup 1.0000x reference)
"""Multi-head self-attention (RoPE, causal) TRN2 Bass kernel — v2.

Problem: B=4, S=2048, D=1024, H=16, Dh=64, fp32 in/out.

Sharding (8 cores): DP=4 over batch x TP=2 over heads (Megatron-style).
Core c handles batch c//2 with heads (c%2)*8 .. (c%2)*8+7 and produces a
partial output [S, D] (stored transposed, bf16); the host sums the two TP
partials per batch (the all-reduce after out_projection).

v2 changes vs v1 (386.7us):
  * All SBUF operands bf16: enables PE Fast-Weight-Load (4x LDWEIGHTS),
    2x DVE elementwise, half DMA / SBUF traffic. PSUM accumulation and
    softmax denominators stay fp32. (rel-err budget is ~67x measured v1.)
  * Globally software-pipelined emission: QKV(ts+1), attention row ts,
    norm(ts-1) and out-projection are interleaved chunk-wise so the
    in-order PE queue always has independent matmuls while ACT (exp) is
    the per-row bottleneck, and HAM never re-throttles.
  * PE + ACT warmup at t=0 (HAM un-throttle + exp table preload) under
    the initial DMA fill; W DMA split column-part-first so the first
    QKV chain unlocks after ~2MB instead of 8MB.
  * Scalar engine runs ONLY exp; copies pinned to DVE, RoPE swaps +
    causal masks + denominator gathers on GpSimd.
  * Norm: per-pair broadcast (one [8x128] one-hot matmul per pair) after
    repacking odd-head ctx, one in-place [128,512] bf16 multiply.
"""

import sys

for _p in ("/opt/trn_rl_repo", "/root/.axon_site/_ro/trn_rl_repo"):
    if _p not in sys.path:
        sys.path.insert(0, _p)

import numpy as np
import ml_dtypes

import concourse.bacc as bacc
import concourse.bass_utils as bass_utils
import concourse.mybir as mybir
import concourse.tile as tile
from concourse.bass_utils import run_bass_kernel_spmd

F32 = mybir.dt.float32
F32R = mybir.dt.float32r
BF16 = mybir.dt.bfloat16
EXP = mybir.ActivationFunctionType.Exp
BF = ml_dtypes.bfloat16

B, S, D = 4, 2048, 1024
H, DH = 16, 64
THETA = 10000.0
NCORES, TP, HLOC = 8, 2, 8          # 8 local heads per core, 4 pairs
NPAIR = HLOC // 2
NT = S // 512                        # 4 q/t tiles of 512
NTQ = S // 128                       # 16 t-chunks of 128
ND = D // 128                        # 8 d-chunks
SCALE = 1.0 / 8.0                    # 1/sqrt(DH)
VW = DH + 2                          # v row stride (64 dv + ones + pad)

_PROGRAM = None


def _merge_emit(*streams):
    """Interleave chunk streams proportionally by estimated ns.

    Each stream is a list of (est_ns, closure). Emission order within a
    stream is preserved; across streams we pace by fraction-completed so
    the instruction queues see a balanced mix.
    """
    streams = [list(s) for s in streams if s]
    totals = [max(1.0, sum(c for c, _ in s)) for s in streams]
    done = [0.0] * len(streams)
    idx = [0] * len(streams)
    while True:
        best, bestf = -1, None
        for k, s in enumerate(streams):
            if idx[k] >= len(s):
                continue
            f = done[k] / totals[k]
            if bestf is None or f < bestf:
                best, bestf = k, f
        if best < 0:
            return
        cost, fn = streams[best][idx[best]]
        fn()
        done[best] += cost
        idx[best] += 1


def _build_program():
    nc = bacc.Bacc(None)

    xT_d = nc.dram_tensor("xT", [D, S], BF16, kind="ExternalInput")
    wqkvT_d = nc.dram_tensor("wqkvT", [D, 3 * HLOC * DH], BF16, kind="ExternalInput")
    woT_d = nc.dram_tensor("woT", [NPAIR, 128, D], BF16, kind="ExternalInput")
    cos_d = nc.dram_tensor("cosT", [128, S], BF16, kind="ExternalInput")
    sin_d = nc.dram_tensor("sinT", [128, S], BF16, kind="ExternalInput")
    tri_d = nc.dram_tensor("tri", [128, 128], BF16, kind="ExternalInput")
    ident_d = nc.dram_tensor("ident", [128, 128], BF16, kind="ExternalInput")
    out_d = nc.dram_tensor("out", [D, S], BF16, kind="ExternalOutput")

    with tile.TileContext(nc) as tc:
        with (
            tc.tile_pool(name="const", bufs=1) as constp,
            tc.tile_pool(name="qkpool", bufs=1) as qkpool,
            tc.tile_pool(name="vpool", bufs=1) as vpool,
            tc.tile_pool(name="wpool", bufs=1) as wpool,
            tc.tile_pool(name="xpool", bufs=1) as xpool,
            tc.tile_pool(name="ropep", bufs=1) as ropep,
            tc.tile_pool(name="ptpool", bufs=1) as ptpool,
            tc.tile_pool(name="nrmpool", bufs=1) as nrmpool,
            tc.tile_pool(name="otpool", bufs=1) as otpool,
            tc.tile_pool(name="wopool", bufs=1) as wopool,
            tc.tile_pool(name="ps_sm", bufs=1, space="PSUM") as ps_sm,
            tc.tile_pool(name="ps_st", bufs=1, space="PSUM") as ps_st,
            tc.tile_pool(name="ps_ctx", bufs=1, space="PSUM") as ps_ctx,
        ):
            # ---------------- warmup: HAM un-throttle + ACT table ----------
            wmov = constp.tile([128, 512], BF16)
            nc.vector.memset(wmov[:], 0.001)
            warm_ps = ps_sm.tile([128, 512], F32, tag="sm", bufs=2, name="warm")
            for k in range(10):
                nc.tensor.matmul(
                    warm_ps[:], wmov[:, (k % 4) * 128:(k % 4 + 1) * 128],
                    wmov[:], start=True, stop=True)
            dumex = constp.tile([128, 32], BF16)
            nc.scalar.activation(dumex[:], wmov[:, 0:32], EXP, scale=0.125)

            # ---------------- persistent tiles ----------------
            qt = [qkpool.tile([128, S], BF16, name=f"qt{p}") for p in range(NPAIR)]
            kt = [qkpool.tile([128, S], BF16, name=f"kt{p}") for p in range(NPAIR)]
            vt = [vpool.tile([128, HLOC, VW], BF16, name=f"v{t}") for t in range(NTQ)]
            w_sb = [wpool.tile([128, 3 * HLOC * DH], BF16, name=f"w{d}") for d in range(ND)]
            cos_sb = constp.tile([128, S], BF16, name="cos")
            sin_sb = constp.tile([128, S], BF16, name="sin")
            tri_sb = constp.tile([128, 128], BF16, name="tri")
            ident_sb = constp.tile([128, 128], BF16, name="ident")
            wo_sb = [wopool.tile([128, D], BF16, name=f"wo{p}") for p in range(NPAIR)]

            xTr = None  # xa tiles come from xpool with tag rotation

            def xa_dma(ts, xa):
                tsl = slice(ts * 512, (ts + 1) * 512)
                for d in range(ND):
                    nc.sync.dma_start(xa[:, d, :], xT_d[d * 128:(d + 1) * 128, tsl])

            # ---------------- initial DMA fill (emission order matters) ----
            xa0 = xpool.tile([128, ND, 512], BF16, tag="x", bufs=2, name="xa0")
            # interleave x(ts=0) with the first 256 W columns so the e=0
            # QKV chain unlocks after ~1.5MB of traffic
            for d in range(ND):
                nc.sync.dma_start(xa0[:, d, :], xT_d[d * 128:(d + 1) * 128, 0:512])
                nc.sync.dma_start(w_sb[d][:, 0:256], wqkvT_d[d * 128:(d + 1) * 128, 0:256])
            for d in range(ND):
                nc.sync.dma_start(w_sb[d][:, 256:512], wqkvT_d[d * 128:(d + 1) * 128, 256:512])
            nc.sync.dma_start(cos_sb[:], cos_d[:])
            nc.sync.dma_start(sin_sb[:], sin_d[:])
            for part in (1, 2):
                psl = slice(part * 512, (part + 1) * 512)
                for d in range(ND):
                    nc.sync.dma_start(w_sb[d][:, psl], wqkvT_d[d * 128:(d + 1) * 128, psl])
            nc.sync.dma_start(tri_sb[:], tri_d[:])
            nc.sync.dma_start(ident_sb[:], ident_d[:])

            # ---------------- chunk generators ----------------
            def qkv_chunks(ts, xa):
                """QKV projection + RoPE for q/t tile ts. ~12 chunks."""
                tsl = slice(ts * 512, (ts + 1) * 512)
                chunks = []

                def qk_chunk(e):
                    def fn(e=e):
                        ps = ps_sm.tile([128, 512], F32, tag="sm", bufs=2)
                        for d in range(ND):
                            nc.tensor.matmul(
                                ps[:], w_sb[d][:, e * 128:(e + 1) * 128],
                                xa[:, d, :],
                                start=(d == 0), stop=(d == ND - 1),
                            )
                        dst = qt[e] if e < NPAIR else kt[e - NPAIR]
                        nc.vector.tensor_copy(dst[:, tsl], ps[:])
                        sw = ropep.tile([128, 512], BF16, tag="sw", bufs=2)
                        for qd in range(4):
                            sq = qd ^ 1
                            nc.gpsimd.dma_start(
                                sw[qd * 32:(qd + 1) * 32, :],
                                dst[sq * 32:(sq + 1) * 32, tsl],
                            )
                        t1 = ropep.tile([128, 512], BF16, tag="t1", bufs=2)
                        nc.vector.tensor_mul(t1[:], dst[:, tsl], cos_sb[:, tsl])
                        nc.vector.tensor_mul(sw[:], sw[:], sin_sb[:, tsl])
                        nc.vector.tensor_add(dst[:, tsl], t1[:], sw[:])
                    return fn

                for e in range(2 * NPAIR):
                    chunks.append((2600.0, qk_chunk(e)))

                def v_chunk(tq0):
                    def fn(tq0=tq0):
                        tq = ts * 4 + tq0
                        psv = ps_sm.tile([128, 512], F32, tag="sm", bufs=2)
                        for d in range(ND):
                            nc.tensor.matmul(
                                psv[:],
                                xa[:, d, tq0 * 128:(tq0 + 1) * 128],
                                w_sb[d][:, 2 * HLOC * DH:3 * HLOC * DH],
                                start=(d == 0), stop=(d == ND - 1),
                            )
                        v = vt[tq]
                        nc.vector.tensor_copy(
                            v[:, :, 0:DH],
                            psv.rearrange("p (h d) -> p h d", h=HLOC),
                        )
                        nc.gpsimd.memset(v[:, :, DH:DH + 1], 1.0)
                    return fn

                for tq0 in range(4):
                    chunks.append((2200.0, v_chunk(tq0)))
                return chunks

            def att_chunks(i):
                """Attention row i, flipped-PV form.

                Per (pair, kv-block j): QK scores (transposed [kv, q]) ->
                exp -> PV with pt as STATIONARY ([128 kv, 128 q] per
                q-subblock qb) and v as MOVING ([128, 65], col 64 = ones):
                out ctxq[q-part, 65] accumulates over j in PSUM. Softmax
                denominators land on the q-partition axis (col 64), so
                normalization is a per-partition tensor_scalar fused into
                the PSUM evacuation; the [q, d] -> [d, q] transpose for the
                out-projection is an N=128 matmul against the identity.
                """
                chunks = []
                nj = 4 * i + 4
                isl = slice(512 * i, 512 * (i + 1))

                for p in range(NPAIR):
                    # two PSUM banks: bank A holds qb 0,1; bank B qb 2,3
                    cq = [ps_ctx.tile([128, 2, 2, DH + 1], F32, tag=f"cq{b}",
                                      bufs=1, name=f"cq{b}_{i}_{p}")
                          for b in range(2)]
                    pt_q = []  # software pipeline: PV for j runs in chunk j+1

                    def pv_emit(p, j, cq, pt):
                        for qb in range(max(0, j - 4 * i), 4):
                            for h in range(2):
                                nc.tensor.matmul(
                                    cq[qb // 2][:, qb % 2, h, :],
                                    pt[:, h, qb * 128:(qb + 1) * 128],
                                    vt[j][:, 2 * p + h, 0:DH + 1],
                                    start=(j == 0 and h == 0 and qb % 2 == 0),
                                    stop=(j == 4 * i + qb and h == 1),
                                    skip_group_check=True,
                                )

                    st_h = []  # row-3 batched-exp: pending 4-bank score tile

                    def j_chunk(p, j, cq):
                        def fn():
                            lo = max(0, 128 * j - 512 * i)
                            diag = lo == 128 * j - 512 * i
                            qsl = slice(512 * i + lo, 512 * (i + 1))
                            ksl = slice(j * 128, (j + 1) * 128)
                            batch2 = i == 3  # ACT-bound row: halve exp count
                            # one 4-bank score tile per pair; halves rotate
                            # per chunk (manual double-buffering via Tile's
                            # range tracking)
                            if j == 0:
                                st_h.append(ps_st.tile(
                                    [128, 2, 2, 512], F32, tag="st",
                                    bufs=1, name="st2"))
                            stv = st_h[0][:, j % 2]
                            if diag:
                                # pre-write -BIG into the masked triangle; the
                                # QK matmul accumulates on top (has_written),
                                # so exp() gives exact zeros — no post-mask.
                                nc.tensor.matmul(
                                    stv[:, 0, lo:lo + 128], tri_sb[:],
                                    ident_sb[:], start=True, stop=False)
                                nc.tensor.matmul(
                                    stv[:, 1, lo:lo + 128], tri_sb[:],
                                    ident_sb[:], start=True, stop=False)
                            nc.tensor.matmul(
                                stv[:, 0, lo:512], kt[p][0:64, ksl],
                                qt[p][0:64, qsl], tile_position=(0, 0),
                                start=not diag, stop=True,
                            )
                            nc.tensor.matmul(
                                stv[:, 1, lo:512], kt[p][64:128, ksl],
                                qt[p][64:128, qsl], tile_position=(64, 0),
                                start=not diag, stop=True,
                            )
                            # PV deferred two chunks (exp ~1.1us needs more
                            # than one chunk of latency hiding)
                            if len(pt_q) >= 2:
                                jd, ptd = pt_q.pop(0)
                                pv_emit(p, jd, cq, ptd)
                            if not batch2:
                                pt = ptpool.tile([128, 2, 512], BF16,
                                                 tag="pt", bufs=6)
                                nc.scalar.activation(
                                    pt[:, :, lo:512], stv[:, :, lo:512], EXP,
                                    scale=SCALE,
                                )
                                pt_q.append((j, pt))
                            elif j % 2 == 1:
                                lo_min = max(0, 128 * (j - 1) - 512 * i)
                                pt2 = ptpool.tile([128, 2, 2, 512], BF16,
                                                  tag="pt2", bufs=3)
                                nc.scalar.activation(
                                    pt2[:, :, :, lo_min:512].rearrange(
                                        "p a b q -> p (a b) q"),
                                    st_h[0][:, :, :, lo_min:512].rearrange(
                                        "p a b q -> p (a b) q"), EXP,
                                    scale=SCALE,
                                )
                                pt_q.append((j - 1, pt2[:, 0]))
                                pt_q.append((j, pt2[:, 1]))
                            if j == nj - 1:
                                st_h.pop()
                        return fn

                    for j in range(nj):
                        lo = max(0, 128 * j - 512 * i)
                        qb0 = max(0, j - 4 * i)
                        chunks.append(((512 - lo) * 0.45 + (4 - qb0) * 70 + 250,
                                       j_chunk(p, j, cq)))

                    def pv_flush(p=p, cq=cq):
                        def fn():
                            if pt_q:
                                jd, ptd = pt_q.pop(0)
                                pv_emit(p, jd, cq, ptd)
                        return fn
                    for _ in range(3 if i == 3 else 2):
                        chunks.append((600.0, pv_flush()))

                    def evac_rcp(p=p, cq=cq, sink=pt_q):
                        rcp = nrmpool.tile([128, 2, 2, 2, 1], F32, tag="rcp",
                                           bufs=2, name=f"rcp{i}_{p}")
                        for b in range(2):
                            nc.vector.reciprocal_approx_fast(
                                rcp[:, b].rearrange("p a b c -> p (a b) c"),
                                cq[b][:, :, :, DH:DH + 1].rearrange(
                                    "p a b c -> p (a b) c"))
                        sink.append(rcp)
                    chunks.append((250.0, evac_rcp))

                    def evac_norm(qb, p=p, cq=cq, src=pt_q):
                        def fn():
                            rcp = src[-1]
                            ctxn = nrmpool.tile([128, 2, DH], BF16, tag="ctxn",
                                                bufs=3, name=f"cn{i}_{p}_{qb}")
                            for h in range(2):
                                nc.vector.tensor_scalar_mul(
                                    ctxn[:, h, :],
                                    cq[qb // 2][:, qb % 2, h, 0:DH],
                                    rcp[:, qb // 2, qb % 2, h],
                                )
                            tp = ps_sm.tile([128, 512], F32, tag="sm", bufs=2,
                                            name=f"tp{i}_{p}_{qb}")
                            nc.tensor.matmul(
                                tp[:, 0:128],
                                ctxn.rearrange("p h d -> p (h d)"),
                                ident_sb[:],
                            )
                            nc.vector.tensor_copy(
                                qt[p][:, 512 * i + 128 * qb:
                                      512 * i + 128 * (qb + 1)],
                                tp[:, 0:128])
                            if qb == 3:
                                src.pop()
                        return fn

                    for qb in range(4):
                        chunks.append((550.0, evac_norm(qb)))
                return chunks

            def out_chunks(ts):
                """Out projection for q/t tile ts (needs norm(ts) done)."""
                tsl = slice(ts * 512, (ts + 1) * 512)
                chunks = []

                def ec_chunk(ec):
                    def fn(ec=ec):
                        ecs = slice(ec * 128, (ec + 1) * 128)
                        pso = ps_sm.tile([128, 512], F32, tag="sm", bufs=2,
                                         name=f"pso{ts}_{ec}")
                        for p in range(NPAIR):
                            nc.tensor.matmul(
                                pso[:], wo_sb[p][:, ecs], qt[p][:, tsl],
                                start=(p == 0), stop=(p == NPAIR - 1),
                            )
                        ot = otpool.tile([128, 512], BF16, tag="ot", bufs=3)
                        nc.vector.tensor_copy(ot[:], pso[:])
                        nc.sync.dma_start(out_d[ecs, tsl], ot[:])
                    return fn

                for ec in range(D // 128):
                    chunks.append((1100.0, ec_chunk(ec)))
                return chunks

            # ---------------- emission schedule ----------------
            xa_t = [xa0, None, None, None]

            def prefetch(ts):
                def fn(ts=ts):
                    xa = xpool.tile([128, ND, 512], BF16, tag="x", bufs=2,
                                    name=f"xa{ts}")
                    xa_t[ts] = xa
                    xa_dma(ts, xa)
                return [(200.0, fn)]

            # QKV(0) sequential (nothing else to overlap yet)
            for _, fn in qkv_chunks(0, xa0):
                fn()
            for _, fn in prefetch(1):
                fn()

            # wo loads: emit after phase-0 DMAs so they don't delay them
            def wo_load():
                for p in range(NPAIR):
                    nc.sync.dma_start(wo_sb[p][:], woT_d[p])

            # round 0: ATT(0) || QKV(1)
            _merge_emit(att_chunks(0),
                        qkv_chunks(1, xa_t[1]) + prefetch(2) + [(200.0, wo_load)])
            # round 1: ATT(1) || QKV(2) || OUT(0)
            _merge_emit(att_chunks(1),
                        qkv_chunks(2, xa_t[2]) + prefetch(3),
                        out_chunks(0))
            # round 2: ATT(2) || QKV(3) || OUT(1)
            _merge_emit(att_chunks(2),
                        qkv_chunks(3, xa_t[3]),
                        out_chunks(1))
            # round 3: ATT(3) || OUT(2)
            _merge_emit(att_chunks(3),
                        out_chunks(2))
            # tail
            for _, fn in out_chunks(3):
                fn()

    nc.compile()
    return nc


def _get_program():
    global _PROGRAM
    if _PROGRAM is None:
        _PROGRAM = _build_program()
    return _PROGRAM


def _prep_in_maps(in_features, token_positions, W_qkv, W_out):
    in_features = np.asarray(in_features, dtype=np.float32)
    token_positions = np.asarray(token_positions)
    W_qkv = np.asarray(W_qkv, dtype=np.float32)
    W_out = np.asarray(W_out, dtype=np.float32)

    # RoPE pair permutation: [x0 of freq 0..31 | x1 of freq 0..31]
    perm = np.concatenate([np.arange(0, DH, 2), np.arange(1, DH, 2)])

    wqkvT, woT = [], []
    for tp in range(TP):
        rows = []
        for sect in range(2):  # Q, K (permuted)
            for h in range(HLOC):
                g = tp * HLOC + h
                rows.append(W_qkv[sect * D + g * DH + perm])
        for h in range(HLOC):  # V natural
            g = tp * HLOC + h
            rows.append(W_qkv[2 * D + g * DH:2 * D + (g + 1) * DH])
        Wl = np.concatenate(rows, axis=0)  # [1536, 1024]
        wqkvT.append(np.ascontiguousarray(Wl.T).astype(BF))
        woT.append(np.ascontiguousarray(np.stack(
            [np.concatenate([
                W_out[:, (tp * HLOC + 2 * p) * DH:(tp * HLOC + 2 * p + 1) * DH].T,
                W_out[:, (tp * HLOC + 2 * p + 1) * DH:(tp * HLOC + 2 * p + 2) * DH].T,
            ], axis=0) for p in range(NPAIR)])).astype(BF))

    half = DH // 2
    inv_freq = (THETA ** (-2.0 * np.arange(half, dtype=np.float32) / DH)).astype(np.float32)
    ang = token_positions.astype(np.float32)[:, None] * inv_freq[None, :]  # [S, 32]
    cos_t = np.cos(ang).T.astype(np.float32)  # [32, S]
    sin_t = np.sin(ang).T.astype(np.float32)
    cos128 = np.ascontiguousarray(np.tile(cos_t, (4, 1))).astype(BF)
    sin128 = np.ascontiguousarray(
        np.tile(np.concatenate([-sin_t, sin_t], axis=0), (2, 1))).astype(BF)
    # tri (as matmul lhsT): tri.T @ I has -BIG at [kv, c] where kv > c
    # (scores stored transposed [kv, q]; strictly-future kv get -BIG)
    tri = np.triu(np.full((128, 128), -30000.0, dtype=np.float32), 1).astype(BF)
    ident = np.eye(128, dtype=np.float32).astype(BF)

    in_maps = []
    for c in range(NCORES):
        b, tp = c // 2, c % 2
        in_maps.append({
            "xT": np.ascontiguousarray(in_features[b].T).astype(BF),
            "wqkvT": wqkvT[tp],
            "woT": woT[tp],
            "cosT": cos128,
            "sinT": sin128,
            "tri": tri,
            "ident": ident,
        })
    return in_maps


def run(in_features, token_positions, W_qkv, W_out, **spmd_kwargs):
    """Run the kernel; returns (output [B,S,D] f32, BassKernelResults)."""
    in_maps = _prep_in_maps(in_features, token_positions, W_qkv, W_out)
    nc = _get_program()
    res = run_bass_kernel_spmd(nc, in_maps, core_ids=list(range(NCORES)), **spmd_kwargs)
    outs = [res.results[c]["out"].astype(np.float32) for c in range(NCORES)]
    full = np.stack([(outs[2 * b] + outs[2 * b + 1]).T for b in range(B)])
    return full.astype(np.float32), res


def kernel(in_features, token_positions, W_qkv, W_out):
    out, _ = run(in_features, token_positions, W_qkv, W_out)
    return out



# revision 35
# speedup vs baseline: 1.5371x; 1.5371x over previous
"""Multi-head self-attention (RoPE, causal) TRN2 Bass kernel — v5.

Problem: B=4, S=2048, D=1024, H=16, Dh=64, fp32 in/out.

Sharding (8 cores): DP=4 over batch x TP=2 over heads (Megatron-style).
Core c handles batch c//2 with heads (c%2)*8 .. (c%2)*8+7 and produces a
partial output [S, D] (stored transposed, bf16); the host sums the two TP
partials per batch (the all-reduce after out_projection).

Design (v2 386.7us -> v5 ~293us measured min-of-3; ~±2% run noise):
  * All SBUF operands bf16 (PE Fast-Weight-Load, 2x DVE, half DMA);
    PSUM accumulation and softmax denominators stay fp32.
  * Causal mask as matmul: a strictly-triangular -30000 stationary x
    identity pre-writes the diagonal score block (start=True), the QK
    matmul accumulates on top via per-element PSUM has_written
    (accumulate-where-written, overwrite-where-fresh); exp then emits
    exact zeros. No post-exp masking, nothing on GpSimd's critical path.
  * Flipped PV: scores pt [kv, q] are the STATIONARY operand, v
    [128, 65] the moving one (col 64 = ones), so ctx accumulates as
    [q-part, 65] per 128-q subblock and softmax denominators land on
    the q-partition axis for free. N=65 matmuls issue at ~32ns.
  * Normalization fused into PSUM evacuation: reciprocal_approx_fast +
    one per-partition tensor_scalar_mul per (qb, head); the [q, d] ->
    [d, q] transpose for the out-projection is an N=128 matmul against
    the identity, landing ctx back in qt[p] for the unchanged
    out-projection chunks.
  * Software pipelining: PV for kv-block j is emitted one chunk later
    (under the next chunk's QK) so exp (~1.1us) is latency-hidden; the
    evacuation is split into rcp/per-qb chunks for the same reason.
  * Globally interleaved emission (_merge_emit): QKV(ts+1), attention
    row ts and out-projection(ts-1) keep the in-order PE queue fed
    while ACT (exp) paces attention; HAM never re-throttles.
  * PSUM (8 banks exactly): proj evac + transposes share tag "sm" (2),
    double-buffered scores (4), ctx accumulators cq (2).
  * PE + ACT warmup at t=0 under the initial DMA fill; W DMA split
    column-part-first so the first QKV chain unlocks after ~2MB.

Measured (NTFF): PE ~245us busy (80%, bottleneck), ACT(exp) ~163us,
DVE ~150us, GpSimd ~99us. Known-structural: the last attention row is
ACT-bound (row-3 exp ~68us vs ~45us of PE work available there).
Failed experiments (do not retry naively): batching exp over j-pairs
with a single-buffered score tile (serializes QK behind exp, +50us);
fp8 projections (est. rel-err ~2e-2 vs the 2e-2 gate).
"""

import sys

for _p in ("/opt/trn_rl_repo", "/root/.axon_site/_ro/trn_rl_repo"):
    if _p not in sys.path:
        sys.path.insert(0, _p)

import numpy as np
import ml_dtypes

import concourse.bacc as bacc
import concourse.bass_utils as bass_utils
import concourse.mybir as mybir
import concourse.tile as tile
from concourse.bass_utils import run_bass_kernel_spmd

F32 = mybir.dt.float32
F32R = mybir.dt.float32r
BF16 = mybir.dt.bfloat16
EXP = mybir.ActivationFunctionType.Exp
BF = ml_dtypes.bfloat16

B, S, D = 4, 2048, 1024
H, DH = 16, 64
THETA = 10000.0
NCORES, TP, HLOC = 8, 2, 8          # 8 local heads per core, 4 pairs
NPAIR = HLOC // 2
NT = S // 512                        # 4 q/t tiles of 512
NTQ = S // 128                       # 16 t-chunks of 128
ND = D // 128                        # 8 d-chunks
SCALE = 1.0 / 8.0                    # 1/sqrt(DH)
VW = DH + 2                          # v row stride (64 dv + ones + pad)

_PROGRAM = None


def _merge_emit(*streams):
    """Interleave chunk streams proportionally by estimated ns.

    Each stream is a list of (est_ns, closure). Emission order within a
    stream is preserved; across streams we pace by fraction-completed so
    the instruction queues see a balanced mix.
    """
    streams = [list(s) for s in streams if s]
    totals = [max(1.0, sum(c for c, _ in s)) for s in streams]
    done = [0.0] * len(streams)
    idx = [0] * len(streams)
    while True:
        best, bestf = -1, None
        for k, s in enumerate(streams):
            if idx[k] >= len(s):
                continue
            f = done[k] / totals[k]
            if bestf is None or f < bestf:
                best, bestf = k, f
        if best < 0:
            return
        cost, fn = streams[best][idx[best]]
        fn()
        done[best] += cost
        idx[best] += 1


def _build_program():
    nc = bacc.Bacc(None)

    xT_d = nc.dram_tensor("xT", [D, S], BF16, kind="ExternalInput")
    wqkvT_d = nc.dram_tensor("wqkvT", [D, 3 * HLOC * DH], BF16, kind="ExternalInput")
    woT_d = nc.dram_tensor("woT", [NPAIR, 128, D], BF16, kind="ExternalInput")
    cos_d = nc.dram_tensor("cosT", [128, S], BF16, kind="ExternalInput")
    sin_d = nc.dram_tensor("sinT", [128, S], BF16, kind="ExternalInput")
    tri_d = nc.dram_tensor("tri", [128, 128], BF16, kind="ExternalInput")
    ident_d = nc.dram_tensor("ident", [128, 128], BF16, kind="ExternalInput")
    out_d = nc.dram_tensor("out", [D, S], BF16, kind="ExternalOutput")

    with tile.TileContext(nc) as tc:
        with (
            tc.tile_pool(name="const", bufs=1) as constp,
            tc.tile_pool(name="qkpool", bufs=1) as qkpool,
            tc.tile_pool(name="vpool", bufs=1) as vpool,
            tc.tile_pool(name="wpool", bufs=1) as wpool,
            tc.tile_pool(name="xpool", bufs=1) as xpool,
            tc.tile_pool(name="ropep", bufs=1) as ropep,
            tc.tile_pool(name="ptpool", bufs=1) as ptpool,
            tc.tile_pool(name="nrmpool", bufs=1) as nrmpool,
            tc.tile_pool(name="otpool", bufs=1) as otpool,
            tc.tile_pool(name="wopool", bufs=1) as wopool,
            tc.tile_pool(name="ps_sm", bufs=1, space="PSUM") as ps_sm,
            tc.tile_pool(name="ps_st", bufs=1, space="PSUM") as ps_st,
            tc.tile_pool(name="ps_ctx", bufs=1, space="PSUM") as ps_ctx,
        ):
            # ---------------- warmup: HAM un-throttle + ACT table ----------
            wmov = constp.tile([128, 512], BF16)
            nc.vector.memset(wmov[:], 0.001)
            warm_ps = ps_sm.tile([128, 512], F32, tag="sm", bufs=2, name="warm")
            for k in range(10):
                nc.tensor.matmul(
                    warm_ps[:], wmov[:, (k % 4) * 128:(k % 4 + 1) * 128],
                    wmov[:], start=True, stop=True)
            dumex = constp.tile([128, 32], BF16)
            nc.scalar.activation(dumex[:], wmov[:, 0:32], EXP, scale=0.125)

            # ---------------- persistent tiles ----------------
            qt = [qkpool.tile([128, S], BF16, name=f"qt{p}") for p in range(NPAIR)]
            kt = [qkpool.tile([128, S], BF16, name=f"kt{p}") for p in range(NPAIR)]
            vt = [vpool.tile([128, HLOC, VW], BF16, name=f"v{t}") for t in range(NTQ)]
            w_sb = [wpool.tile([128, 3 * HLOC * DH], BF16, name=f"w{d}") for d in range(ND)]
            cos_sb = constp.tile([128, S], BF16, name="cos")
            sin_sb = constp.tile([128, S], BF16, name="sin")
            tri_sb = constp.tile([128, 128], BF16, name="tri")
            ident_sb = constp.tile([128, 128], BF16, name="ident")
            wo_sb = [wopool.tile([128, D], BF16, name=f"wo{p}") for p in range(NPAIR)]

            xTr = None  # xa tiles come from xpool with tag rotation

            def xa_dma(ts, xa):
                tsl = slice(ts * 512, (ts + 1) * 512)
                for d in range(ND):
                    nc.sync.dma_start(xa[:, d, :], xT_d[d * 128:(d + 1) * 128, tsl])

            # ---------------- initial DMA fill (emission order matters) ----
            xa0 = xpool.tile([128, ND, 512], BF16, tag="x", bufs=3, name="xa0")
            # interleave x(ts=0) with W column-parts so the e=0 chain
            # unlocks after ~2MB of traffic
            for d in range(ND):
                nc.sync.dma_start(xa0[:, d, :], xT_d[d * 128:(d + 1) * 128, 0:512])
                nc.sync.dma_start(w_sb[d][:, 0:512], wqkvT_d[d * 128:(d + 1) * 128, 0:512])
            nc.sync.dma_start(cos_sb[:], cos_d[:])
            nc.sync.dma_start(sin_sb[:], sin_d[:])
            for part in (1, 2):
                psl = slice(part * 512, (part + 1) * 512)
                for d in range(ND):
                    nc.sync.dma_start(w_sb[d][:, psl], wqkvT_d[d * 128:(d + 1) * 128, psl])
            nc.sync.dma_start(tri_sb[:], tri_d[:])
            nc.sync.dma_start(ident_sb[:], ident_d[:])

            # ---------------- chunk generators ----------------
            def qkv_chunks(ts, xa):
                """QKV projection + RoPE for q/t tile ts. ~12 chunks."""
                tsl = slice(ts * 512, (ts + 1) * 512)
                chunks = []

                def qk_chunk(e):
                    def fn(e=e):
                        ps = ps_sm.tile([128, 512], F32, tag="sm", bufs=2)
                        for d in range(ND):
                            nc.tensor.matmul(
                                ps[:], w_sb[d][:, e * 128:(e + 1) * 128],
                                xa[:, d, :],
                                start=(d == 0), stop=(d == ND - 1),
                            )
                        dst = qt[e] if e < NPAIR else kt[e - NPAIR]
                        nc.vector.tensor_copy(dst[:, tsl], ps[:])
                        sw = ropep.tile([128, 512], BF16, tag="sw", bufs=4)
                        for qd in range(4):
                            sq = qd ^ 1
                            nc.gpsimd.dma_start(
                                sw[qd * 32:(qd + 1) * 32, :],
                                dst[sq * 32:(sq + 1) * 32, tsl],
                            )
                        t1 = ropep.tile([128, 512], BF16, tag="t1", bufs=4)
                        nc.vector.tensor_mul(t1[:], dst[:, tsl], cos_sb[:, tsl])
                        nc.vector.tensor_mul(sw[:], sw[:], sin_sb[:, tsl])
                        nc.vector.tensor_add(dst[:, tsl], t1[:], sw[:])
                    return fn

                for e in range(2 * NPAIR):
                    chunks.append((2600.0, qk_chunk(e)))

                def v_chunk(tq0):
                    def fn(tq0=tq0):
                        tq = ts * 4 + tq0
                        psv = ps_sm.tile([128, 512], F32, tag="sm", bufs=2)
                        for d in range(ND):
                            nc.tensor.matmul(
                                psv[:],
                                xa[:, d, tq0 * 128:(tq0 + 1) * 128],
                                w_sb[d][:, 2 * HLOC * DH:3 * HLOC * DH],
                                start=(d == 0), stop=(d == ND - 1),
                            )
                        v = vt[tq]
                        nc.vector.tensor_copy(
                            v[:, :, 0:DH],
                            psv.rearrange("p (h d) -> p h d", h=HLOC),
                        )
                        nc.gpsimd.memset(v[:, :, DH:DH + 1], 1.0)
                    return fn

                for tq0 in range(4):
                    chunks.append((2200.0, v_chunk(tq0)))
                return chunks

            def att_chunks(i):
                """Attention row i, flipped-PV form.

                Per (pair, kv-block j): QK scores (transposed [kv, q]) ->
                exp -> PV with pt as STATIONARY ([128 kv, 128 q] per
                q-subblock qb) and v as MOVING ([128, 65], col 64 = ones):
                out ctxq[q-part, 65] accumulates over j in PSUM. Softmax
                denominators land on the q-partition axis (col 64), so
                normalization is a per-partition tensor_scalar fused into
                the PSUM evacuation; the [q, d] -> [d, q] transpose for the
                out-projection is an N=128 matmul against the identity.
                """
                chunks = []
                nj = 4 * i + 4
                isl = slice(512 * i, 512 * (i + 1))

                for p in range(NPAIR):
                    # two PSUM banks: bank A holds qb 0,1; bank B qb 2,3
                    cq = [ps_ctx.tile([128, 2, 2, DH + 1], F32, tag=f"cq{b}",
                                      bufs=1, name=f"cq{b}_{i}_{p}")
                          for b in range(2)]
                    pt_q = []  # software pipeline: PV for j runs in chunk j+1

                    def pv_emit(p, j, cq, pt):
                        for qb in range(max(0, j - 4 * i), 4):
                            for h in range(2):
                                nc.tensor.matmul(
                                    cq[qb // 2][:, qb % 2, h, :],
                                    pt[:, h, qb * 128:(qb + 1) * 128],
                                    vt[j][:, 2 * p + h, 0:DH + 1],
                                    start=(j == 0 and h == 0 and qb % 2 == 0),
                                    stop=(j == 4 * i + qb and h == 1),
                                    skip_group_check=True,
                                )

                    def j_chunk(p, j, cq):
                        def fn():
                            lo = max(0, 128 * j - 512 * i)
                            diag = lo == 128 * j - 512 * i
                            qsl = slice(512 * i + lo, 512 * (i + 1))
                            ksl = slice(j * 128, (j + 1) * 128)
                            st = ps_st.tile([128, 2, 512], F32, tag="st", bufs=2)
                            if diag:
                                # pre-write -BIG into the masked triangle; the
                                # QK matmul accumulates on top (has_written),
                                # so exp() gives exact zeros — no post-mask.
                                nc.tensor.matmul(
                                    st[:, 0, lo:lo + 128], tri_sb[:],
                                    ident_sb[:], start=True, stop=False)
                                nc.tensor.matmul(
                                    st[:, 1, lo:lo + 128], tri_sb[:],
                                    ident_sb[:], start=True, stop=False)
                            nc.tensor.matmul(
                                st[:, 0, lo:512], kt[p][0:64, ksl],
                                qt[p][0:64, qsl], tile_position=(0, 0),
                                start=not diag, stop=True,
                            )
                            nc.tensor.matmul(
                                st[:, 1, lo:512], kt[p][64:128, ksl],
                                qt[p][64:128, qsl], tile_position=(64, 0),
                                start=not diag, stop=True,
                            )
                            # PV for the PREVIOUS j (its exp had a full chunk
                            # of latency hiding behind this chunk's QK)
                            if pt_q:
                                pv_emit(p, j - 1, cq, pt_q.pop())
                            pt = ptpool.tile([128, 2, 512], BF16, tag="pt", bufs=10)
                            nc.scalar.activation(
                                pt[:, :, lo:512], st[:, :, lo:512], EXP,
                                scale=SCALE,
                            )
                            pt_q.append(pt)
                        return fn

                    for j in range(nj):
                        lo = max(0, 128 * j - 512 * i)
                        qb0 = max(0, j - 4 * i)
                        chunks.append(((512 - lo) * 0.45 + (4 - qb0) * 70 + 250,
                                       j_chunk(p, j, cq)))

                    def pv_flush(p=p, cq=cq):
                        pv_emit(p, nj - 1, cq, pt_q.pop())
                    chunks.append((600.0, pv_flush))

                    def evac_rcp(p=p, cq=cq, sink=pt_q):
                        rcp = nrmpool.tile([128, 2, 2, 2, 1], F32, tag="rcp",
                                           bufs=2, name=f"rcp{i}_{p}")
                        for b in range(2):
                            nc.vector.reciprocal_approx_fast(
                                rcp[:, b].rearrange("p a b c -> p (a b) c"),
                                cq[b][:, :, :, DH:DH + 1].rearrange(
                                    "p a b c -> p (a b) c"))
                        sink.append(rcp)
                    chunks.append((250.0, evac_rcp))

                    def evac_norm(qb, p=p, cq=cq, src=pt_q):
                        def fn():
                            rcp = src[-1]
                            ctxn = nrmpool.tile([128, 2, DH], BF16, tag="ctxn",
                                                bufs=4, name=f"cn{i}_{p}_{qb}")
                            for h in range(2):
                                nc.vector.tensor_scalar_mul(
                                    ctxn[:, h, :],
                                    cq[qb // 2][:, qb % 2, h, 0:DH],
                                    rcp[:, qb // 2, qb % 2, h],
                                )
                            tp = ps_sm.tile([128, 512], F32, tag="sm", bufs=2,
                                            name=f"tp{i}_{p}_{qb}")
                            nc.tensor.matmul(
                                tp[:, 0:128],
                                ctxn.rearrange("p h d -> p (h d)"),
                                ident_sb[:],
                            )
                            nc.vector.tensor_copy(
                                qt[p][:, 512 * i + 128 * qb:
                                      512 * i + 128 * (qb + 1)],
                                tp[:, 0:128])
                            if qb == 3:
                                src.pop()
                        return fn

                    for qb in range(4):
                        chunks.append((550.0, evac_norm(qb)))
                return chunks

            def out_chunks(ts):
                """Out projection for q/t tile ts (needs norm(ts) done)."""
                tsl = slice(ts * 512, (ts + 1) * 512)
                chunks = []

                def ec_chunk(ec):
                    def fn(ec=ec):
                        ecs = slice(ec * 128, (ec + 1) * 128)
                        pso = ps_sm.tile([128, 512], F32, tag="sm", bufs=2,
                                         name=f"pso{ts}_{ec}")
                        for p in range(NPAIR):
                            nc.tensor.matmul(
                                pso[:], wo_sb[p][:, ecs], qt[p][:, tsl],
                                start=(p == 0), stop=(p == NPAIR - 1),
                            )
                        ot = otpool.tile([128, 512], BF16, tag="ot", bufs=4)
                        nc.vector.tensor_copy(ot[:], pso[:])
                        nc.sync.dma_start(out_d[ecs, tsl], ot[:])
                    return fn

                for ec in range(D // 128):
                    chunks.append((1100.0, ec_chunk(ec)))
                return chunks

            # ---------------- emission schedule ----------------
            xa_t = [xa0, None, None, None]

            def prefetch(ts):
                def fn(ts=ts):
                    xa = xpool.tile([128, ND, 512], BF16, tag="x", bufs=3,
                                    name=f"xa{ts}")
                    xa_t[ts] = xa
                    xa_dma(ts, xa)
                return [(200.0, fn)]

            # QKV(0): qk chunks sequential (DMA-gated anyway), then start
            # attention row 0 under the v chunks. Ordering constraint:
            # v_chunk(j) must be EMITTED before the att chunk whose
            # (one-deferred) PV reads vt[j].
            c0 = qkv_chunks(0, xa0)
            for _, fn in c0[:8]:
                fn()
            for _, fn in prefetch(1):
                fn()
            att0 = att_chunks(0)
            for _, fn in (c0[8], att0[0], att0[1], c0[9], att0[2],
                          c0[10], att0[3], c0[11]):
                fn()

            # wo loads: emit after phase-0 DMAs so they don't delay them
            def wo_load():
                for p in range(NPAIR):
                    nc.sync.dma_start(wo_sb[p][:], woT_d[p])

            # round 0: rest of ATT(0) || QKV(1)
            _merge_emit(att0[4:],
                        qkv_chunks(1, xa_t[1]) + prefetch(2) + [(200.0, wo_load)])
            # round 1: ATT(1) || QKV(2) || OUT(0)
            _merge_emit(att_chunks(1),
                        qkv_chunks(2, xa_t[2]) + prefetch(3),
                        out_chunks(0))
            # round 2: ATT(2) || QKV(3) || OUT(1)
            _merge_emit(att_chunks(2),
                        qkv_chunks(3, xa_t[3]),
                        out_chunks(1))
            # round 3: ATT(3) || OUT(2)
            _merge_emit(att_chunks(3),
                        out_chunks(2))
            # tail
            for _, fn in out_chunks(3):
                fn()

    nc.compile()
    return nc


def _get_program():
    global _PROGRAM
    if _PROGRAM is None:
        _PROGRAM = _build_program()
    return _PROGRAM


def _prep_in_maps(in_features, token_positions, W_qkv, W_out):
    in_features = np.asarray(in_features, dtype=np.float32)
    token_positions = np.asarray(token_positions)
    W_qkv = np.asarray(W_qkv, dtype=np.float32)
    W_out = np.asarray(W_out, dtype=np.float32)

    # RoPE pair permutation: [x0 of freq 0..31 | x1 of freq 0..31]
    perm = np.concatenate([np.arange(0, DH, 2), np.arange(1, DH, 2)])

    wqkvT, woT = [], []
    for tp in range(TP):
        rows = []
        for sect in range(2):  # Q, K (permuted)
            for h in range(HLOC):
                g = tp * HLOC + h
                rows.append(W_qkv[sect * D + g * DH + perm])
        for h in range(HLOC):  # V natural
            g = tp * HLOC + h
            rows.append(W_qkv[2 * D + g * DH:2 * D + (g + 1) * DH])
        Wl = np.concatenate(rows, axis=0)  # [1536, 1024]
        wqkvT.append(np.ascontiguousarray(Wl.T).astype(BF))
        woT.append(np.ascontiguousarray(np.stack(
            [np.concatenate([
                W_out[:, (tp * HLOC + 2 * p) * DH:(tp * HLOC + 2 * p + 1) * DH].T,
                W_out[:, (tp * HLOC + 2 * p + 1) * DH:(tp * HLOC + 2 * p + 2) * DH].T,
            ], axis=0) for p in range(NPAIR)])).astype(BF))

    half = DH // 2
    inv_freq = (THETA ** (-2.0 * np.arange(half, dtype=np.float32) / DH)).astype(np.float32)
    ang = token_positions.astype(np.float32)[:, None] * inv_freq[None, :]  # [S, 32]
    cos_t = np.cos(ang).T.astype(np.float32)  # [32, S]
    sin_t = np.sin(ang).T.astype(np.float32)
    cos128 = np.ascontiguousarray(np.tile(cos_t, (4, 1))).astype(BF)
    sin128 = np.ascontiguousarray(
        np.tile(np.concatenate([-sin_t, sin_t], axis=0), (2, 1))).astype(BF)
    # tri (as matmul lhsT): tri.T @ I has -BIG at [kv, c] where kv > c
    # (scores stored transposed [kv, q]; strictly-future kv get -BIG)
    tri = np.triu(np.full((128, 128), -30000.0, dtype=np.float32), 1).astype(BF)
    ident = np.eye(128, dtype=np.float32).astype(BF)

    in_maps = []
    for c in range(NCORES):
        b, tp = c // 2, c % 2
        in_maps.append({
            "xT": np.ascontiguousarray(in_features[b].T).astype(BF),
            "wqkvT": wqkvT[tp],
            "woT": woT[tp],
            "cosT": cos128,
            "sinT": sin128,
            "tri": tri,
            "ident": ident,
        })
    return in_maps


def run(in_features, token_positions, W_qkv, W_out, **spmd_kwargs):
    """Run the kernel; returns (output [B,S,D] f32, BassKernelResults)."""
    in_maps = _prep_in_maps(in_features, token_positions, W_qkv, W_out)
    nc = _get_program()
    res = run_bass_kernel_spmd(nc, in_maps, core_ids=list(range(NCORES)), **spmd_kwargs)
    outs = [res.results[c]["out"].astype(np.float32) for c in range(NCORES)]
    full = np.stack([(outs[2 * b] + outs[2 * b + 1]).T for b in range(B)])
    return full.astype(np.float32), res


def kernel(in_features, token_positions, W_qkv, W_out):
    out, _ = run(in_features, token_positions, W_qkv, W_out)
    return out

